# revision 7
# baseline (speedup 1.0000x reference)
"""MoE layer kernel for 8 Trainium2 NeuronCores.

Strategy (phase 1, dense expert-parallel):
  - Gate network data-parallel: core c computes gate probs for tokens
    [c*1024, (c+1)*1024) in fp32 exactly (x transposed via PE), AllGather ->
    every core has full gate_probs.
  - Experts expert-parallel: core c runs expert c densely over all 8192
    tokens in bf16 (fp32 accumulate), scales by its combine weight column,
    writes a feature-major partial output yT [O, B].
  - Host: sum partials over cores, transpose -> [B, O]; concat gate prob
    shards; lb_loss computed on device (core 0).
"""

import sys

for _p in ("/opt/trn_rl_repo",):
    if _p not in sys.path:
        sys.path.insert(0, _p)

import numpy as np

import concourse.mybir as mybir
import concourse.tile as tile
from concourse import bacc, bass_utils
from concourse.masks import make_identity

B, D, H, O, E, TOPK = 8192, 1024, 2048, 1024, 8, 2
NC = 8
TB = 256  # token block
NBLK = B // TB  # 32
BS = B // NC  # tokens per core for the gate = 1024
GBLK = BS // TB  # 4

f32 = mybir.dt.float32
bf16 = mybir.dt.bfloat16
Relu = mybir.ActivationFunctionType.Relu
Exp = mybir.ActivationFunctionType.Exp
Ln = mybir.ActivationFunctionType.Ln
Copy = mybir.ActivationFunctionType.Copy
AX = mybir.AxisListType.X

KD = D // 128  # 8   k-tiles of D
MH = H // 128  # 16  m-tiles of H
MO = O // 128  # 8   m-tiles of O

LOG8 = float(np.log(8.0))
MAX_ENT = 8.0 * LOG8


def build_moe(nc, tc):
    dt = nc.dram_tensor
    x_d = dt("x", (B, D), f32, kind="ExternalInput").ap()
    xs_d = dt("xs", (BS, D), f32, kind="ExternalInput").ap()
    gw1_d = dt("gw1", (D, 256), f32, kind="ExternalInput").ap()
    gb1_d = dt("gb1", (256, 1), f32, kind="ExternalInput").ap()
    gw2_d = dt("gw2", (256, 128), f32, kind="ExternalInput").ap()
    gb2_d = dt("gb2", (128, 1), f32, kind="ExternalInput").ap()
    gw3_d = dt("gw3", (128, E), f32, kind="ExternalInput").ap()
    gb3_d = dt("gb3", (E, 1), f32, kind="ExternalInput").ap()
    ew1_d = dt("ew1", (D, H), f32, kind="ExternalInput").ap()
    eb1_d = dt("eb1", (H, 1), f32, kind="ExternalInput").ap()
    ew2_d = dt("ew2", (H, H), f32, kind="ExternalInput").ap()
    eb2_d = dt("eb2", (H, 1), f32, kind="ExternalInput").ap()
    ew3_d = dt("ew3", (H, O), f32, kind="ExternalInput").ap()
    eb3_d = dt("eb3", (O, 1), f32, kind="ExternalInput").ap()
    eoh_d = dt("eoh", (128, E), f32, kind="ExternalInput").ap()

    yT_d = dt("yT", (O, B), f32, kind="ExternalOutput").ap()
    gp_d = dt("gp", (BS, E), f32, kind="ExternalOutput").ap()
    lb_d = dt("lb", (1, 1), f32, kind="ExternalOutput").ap()

    with (
        tc.tile_pool(name="wpool", bufs=1) as wpool,
        tc.tile_pool(name="stage", bufs=2) as stage,
        tc.tile_pool(name="xtok", bufs=2) as xtok_p,
        tc.tile_pool(name="const", bufs=1) as const,
        tc.tile_pool(name="xt", bufs=3) as xt,
        tc.tile_pool(name="xbf", bufs=1) as xbf,
        tc.tile_pool(name="hpool", bufs=1) as hpool,
        tc.tile_pool(name="gpool", bufs=1) as gpool,
        tc.tile_pool(name="small", bufs=4) as small,
        tc.tile_pool(name="ypool", bufs=3) as ypool,
        tc.tile_pool(name="mm", bufs=4, space="PSUM") as mmp,
        tc.tile_pool(name="gps", bufs=2, space="PSUM") as gpsp,
        tc.tile_pool(name="tp", bufs=2, space="PSUM") as tpp,
        tc.tile_pool(name="dram", bufs=1, space="DRAM") as dram,
    ):
        # ---------------- constants ----------------
        ident = const.tile([128, 128], f32)
        make_identity(nc, ident[:])
        ones = const.tile([128, 1], f32)
        nc.vector.memset(ones[:], 1.0)
        eoh = const.tile([128, E], f32)
        nc.sync.dma_start(out=eoh[:], in_=eoh_d)
        ones_row = const.tile([1, 128], f32)
        nc.vector.memset(ones_row[:], 1.0)
        usage = const.tile([1, E], f32)
        nc.vector.memset(usage[:], 0.0)
        eps_c = const.tile([1, 1], f32)
        nc.vector.memset(eps_c[:], 1e-8)

        # ---------------- expert weights -> bf16 resident ----------------
        w1 = wpool.tile([128, KD, H], bf16)
        w2 = wpool.tile([128, MH, H], bf16)
        w3 = wpool.tile([128, MH, O], bf16)
        def load_w_bf16(dst, src_d, k, width):
            for h in range(0, width, 1024):
                st = stage.tile([128, 1024], f32, tag="stage")
                nc.sync.dma_start(
                    out=st[:], in_=src_d[k * 128 : (k + 1) * 128, h : h + 1024]
                )
                nc.vector.tensor_copy(dst[:, k, h : h + 1024], st[:])

        for k in range(KD):
            load_w_bf16(w1, ew1_d, k, H)
        for k in range(MH):
            load_w_bf16(w2, ew2_d, k, H)
        for k in range(MH):
            load_w_bf16(w3, ew3_d, k, O)

        # gate weights fp32 resident
        g1w = wpool.tile([128, KD, 256], f32)
        for k in range(KD):
            nc.sync.dma_start(out=g1w[:, k, :], in_=gw1_d[k * 128 : (k + 1) * 128, :])
        g2w = wpool.tile([128, 2, 128], f32)
        for k in range(2):
            nc.sync.dma_start(out=g2w[:, k, :], in_=gw2_d[k * 128 : (k + 1) * 128, :])
        g3w = wpool.tile([128, E], f32)
        nc.sync.dma_start(out=g3w[:], in_=gw3_d)

        # biases as per-partition columns
        def load_bias_cols(dram_ap, n_m, nm):
            t = const.tile([128, n_m], f32, tag=f"bias_{nm}")
            nc.sync.dma_start(
                out=t[:], in_=dram_ap.rearrange("(m p) o -> p (m o)", p=128)
            )
            return t

        eb1s = load_bias_cols(eb1_d, MH, "eb1")
        eb2s = load_bias_cols(eb2_d, MH, "eb2")
        eb3s = load_bias_cols(eb3_d, MO, "eb3")
        gb1s = load_bias_cols(gb1_d, 2, "gb1")
        gb2s = load_bias_cols(gb2_d, 1, "gb2")
        gb3s = const.tile([E, 1], f32)
        nc.sync.dma_start(out=gb3s[:], in_=gb3_d)

        xbf_dram = dram.tile([B, D], bf16)  # bf16 copy of x
        gpl = dram.tile([BS, E], f32)  # gate probs local (collective in)
        gpa = dram.tile([B, E], f32)  # gate probs allgathered

        # ---------------- cast pass: x -> bf16 DRAM ----------------
        for i in range(B // 128):
            st = stage.tile([128, D], f32, tag="stage")
            nc.sync.dma_start(out=st[:, :D], in_=x_d[i * 128 : (i + 1) * 128, :])
            cb = stage.tile([128, D], bf16, tag="stage")
            nc.vector.tensor_copy(cb[:], st[:, :D])
            nc.sync.dma_start(out=xbf_dram[i * 128 : (i + 1) * 128, :], in_=cb[:])

        # ---------------- gate phase (core's token slice, fp32) ----------------
        for blk in range(GBLK):
            t0 = blk * TB
            xtoks = []
            for j in range(TB // 128):
                xj = xtok_p.tile([128, D], f32, tag="xtok")
                nc.sync.dma_start(out=xj[:], in_=xs_d[t0 + j * 128 : t0 + (j + 1) * 128, :])
                xtoks.append(xj)
            ps0 = gpsp.tile([128, TB], f32, tag="gps")
            ps1 = gpsp.tile([128, TB], f32, tag="gps")
            for k in range(KD):
                xk32 = xt.tile([128, TB], f32, tag="xt32")
                for j in range(TB // 128):
                    tps = tpp.tile([128, 128], f32, tag="tp")
                    nc.tensor.transpose(
                        tps[:], xtoks[j][:, k * 128 : (k + 1) * 128], ident[:]
                    )
                    nc.vector.tensor_copy(xk32[:, j * 128 : (j + 1) * 128], tps[:])
                nc.tensor.matmul(
                    ps0[:], g1w[:, k, 0:128], xk32[:],
                    start=(k == 0), stop=(k == KD - 1),
                )
                nc.tensor.matmul(
                    ps1[:], g1w[:, k, 128:256], xk32[:],
                    start=(k == 0), stop=(k == KD - 1),
                )
            g1 = gpool.tile([128, 2, TB], f32, tag="g1")
            nc.scalar.activation(g1[:, 0, :], ps0[:], Relu, bias=gb1s[:, 0:1])
            nc.scalar.activation(g1[:, 1, :], ps1[:], Relu, bias=gb1s[:, 1:2])
            ps2 = mmp.tile([128, TB], f32, tag="mm")
            nc.tensor.matmul(ps2[:], g2w[:, 0, :], g1[:, 0, :], start=True, stop=False)
            nc.tensor.matmul(ps2[:], g2w[:, 1, :], g1[:, 1, :], start=False, stop=True)
            g2 = gpool.tile([128, TB], f32, tag="g2")
            nc.scalar.activation(g2[:], ps2[:], Relu, bias=gb2s[:, 0:1])
            ps3 = mmp.tile([E, TB], f32, tag="mm")
            nc.tensor.matmul(ps3[:], g3w[:], g2[:], start=True, stop=True)
            logits = gpool.tile([E, TB], f32, tag="logits")
            nc.vector.tensor_scalar_add(logits[:], ps3[:], gb3s[:, 0:1])

            for j in range(TB // 128):
                tp = tpp.tile([128, E], f32, tag="tp")
                nc.tensor.transpose(
                    tp[:], logits[:, j * 128 : (j + 1) * 128], ident[0:E, 0:E]
                )
                lgt = small.tile([128, E], f32, tag="lgt")
                nc.vector.tensor_copy(lgt[:], tp[:])
                s8 = small.tile([128, 8], f32, tag="s8")
                nc.vector.max(s8[:], lgt[:])
                nm = small.tile([128, 1], f32, tag="nm")
                nc.scalar.mul(nm[:], s8[:, 0:1], -1.0)
                ex = small.tile([128, E], f32, tag="ex")
                nc.scalar.activation(ex[:], lgt[:], Exp, bias=nm[:, 0:1])
                ssum = small.tile([128, 1], f32, tag="ssum")
                nc.vector.reduce_sum(ssum[:], ex[:], axis=AX)
                rs = small.tile([128, 1], f32, tag="rs")
                nc.vector.reciprocal(rs[:], ssum[:])
                pt = small.tile([128, E], f32, tag="pt")
                nc.vector.tensor_scalar_mul(pt[:], ex[:], rs[:, 0:1])
                r0 = t0 + j * 128
                nc.sync.dma_start(out=gp_d[r0 : r0 + 128, :], in_=pt[:])
                nc.sync.dma_start(out=gpl[r0 : r0 + 128, :], in_=pt[:])

        nc.gpsimd.collective_compute(
            "AllGather",
            mybir.AluOpType.bypass,
            replica_groups=[list(range(NC))],
            ins=[gpl[:].opt()],
            outs=[gpa[:].opt()],
        )

        # ---------------- expert phase (all tokens, bf16) ----------------
        for blk in range(NBLK):
            t0 = blk * TB
            # combine-weight row for this block (+ usage accumulation)
            ccps = tpp.tile([1, TB], f32, tag="tp")
            for j in range(TB // 128):
                r0 = t0 + j * 128
                pt = small.tile([128, E], f32, tag="cpt")
                nc.sync.dma_start(out=pt[:], in_=gpa[r0 : r0 + 128, :])
                s8 = small.tile([128, 8], f32, tag="cs8")
                nc.vector.max(s8[:], pt[:])
                den = small.tile([128, 1], f32, tag="cden")
                nc.vector.tensor_add(den[:], s8[:, 0:1], s8[:, 1:2])
                rden = small.tile([128, 1], f32, tag="crden")
                nc.vector.reciprocal(rden[:], den[:])
                mask = small.tile([128, E], f32, tag="cmask")
                nc.vector.tensor_scalar(
                    out=mask[:], in0=pt[:], scalar1=s8[:, 1:2], scalar2=None,
                    op0=mybir.AluOpType.is_ge,
                )
                cw = small.tile([128, E], f32, tag="ccw")
                nc.vector.tensor_mul(cw[:], pt[:], mask[:])
                nc.vector.tensor_scalar_mul(cw[:], cw[:], rden[:, 0:1])
                nc.vector.tensor_mul(cw[:], cw[:], eoh[:])
                cc = small.tile([128, 1], f32, tag="ccc")
                nc.vector.reduce_sum(cc[:], cw[:], axis=AX)
                nc.tensor.transpose(ccps[0:1, j * 128 : (j + 1) * 128], cc[:], ident[:])
                ups = tpp.tile([1, E], f32, tag="tp")
                nc.tensor.matmul(ups[:], ones[:], pt[:], start=True, stop=True)
                nc.vector.tensor_add(usage[:], usage[:], ups[:])
            ccrow = small.tile([1, TB], f32, tag="ccrow")
            nc.vector.tensor_copy(ccrow[:], ccps[:])
            ccrep_ps = tpp.tile([128, TB], f32, tag="tp")
            nc.tensor.matmul(ccrep_ps[:], ones_row[:], ccrow[:], start=True, stop=True)
            ccrep = small.tile([128, TB], f32, tag="ccrep")
            nc.vector.tensor_copy(ccrep[:], ccrep_ps[:])

            # x^T block (bf16) via xbar transpose from xbf_dram
            xb = xbf.tile([128, KD, TB], bf16, tag="xb")
            for k in range(KD):
                nc.sync.dma_start(
                    out=xb[:, k, :],
                    in_=xbf_dram[t0 : t0 + TB, k * 128 : (k + 1) * 128],
                    transpose=True,
                )

            h1 = hpool.tile([128, MH, TB], bf16, tag="h1")
            for m in range(MH):
                ps = mmp.tile([128, TB], f32, tag="mm")
                for k in range(KD):
                    nc.tensor.matmul(
                        ps[:], w1[:, k, m * 128 : (m + 1) * 128], xb[:, k, :],
                        start=(k == 0), stop=(k == KD - 1),
                    )
                nc.scalar.activation(h1[:, m, :], ps[:], Relu, bias=eb1s[:, m : m + 1])
            h2 = hpool.tile([128, MH, TB], bf16, tag="h2")
            for m in range(MH):
                ps = mmp.tile([128, TB], f32, tag="mm")
                for k in range(MH):
                    nc.tensor.matmul(
                        ps[:], w2[:, k, m * 128 : (m + 1) * 128], h1[:, k, :],
                        start=(k == 0), stop=(k == MH - 1),
                    )
                nc.scalar.activation(h2[:, m, :], ps[:], Relu, bias=eb2s[:, m : m + 1])
            for m in range(MO):
                ps = mmp.tile([128, TB], f32, tag="mm")
                for k in range(MH):
                    nc.tensor.matmul(
                        ps[:], w3[:, k, m * 128 : (m + 1) * 128], h2[:, k, :],
                        start=(k == 0), stop=(k == MH - 1),
                    )
                yt = ypool.tile([128, TB], f32, tag="yt")
                nc.vector.tensor_scalar_add(yt[:], ps[:], eb3s[:, m : m + 1])
                nc.vector.tensor_mul(yt[:], yt[:], ccrep[:])
                nc.sync.dma_start(
                    out=yT_d[m * 128 : (m + 1) * 128, t0 : t0 + TB], in_=yt[:]
                )

        # ---------------- lb loss ----------------
        u = small.tile([1, E], f32, tag="u")
        nc.scalar.mul(u[:], usage[:], 1.0 / B)
        lu = small.tile([1, E], f32, tag="lu")
        nc.scalar.activation(lu[:], u[:], Ln, bias=eps_c[:, 0:1])
        kls = small.tile([1, 1], f32, tag="kls")
        nc.vector.reduce_sum(kls[:], lu[:], axis=AX)
        kl = small.tile([1, 1], f32, tag="kl")
        # kl = log(1/8) - mean(log(u))
        nc.scalar.activation(kl[:], kls[:], Copy, bias=-LOG8, scale=-1.0 / E)
        # ent = -sum(u * log u); contribution 0.2*0.01*(max_ent - ent)
        ulu = small.tile([1, E], f32, tag="ulu")
        nc.vector.tensor_mul(ulu[:], u[:], lu[:])
        es = small.tile([1, 1], f32, tag="es")
        nc.vector.reduce_sum(es[:], ulu[:], axis=AX)
        # var (ddof=1)
        usum = small.tile([1, 1], f32, tag="usum")
        nc.vector.reduce_sum(usum[:], u[:], axis=AX)
        um = small.tile([1, 1], f32, tag="um")
        nc.scalar.mul(um[:], usum[:], 1.0 / E)
        dv = small.tile([1, E], f32, tag="dv")
        nc.vector.tensor_scalar_sub(dv[:], u[:], um[:, 0:1])
        nc.vector.tensor_mul(dv[:], dv[:], dv[:])
        vs = small.tile([1, 1], f32, tag="vs")
        nc.vector.reduce_sum(vs[:], dv[:], axis=AX)

        a1 = small.tile([1, 1], f32, tag="a1")
        nc.scalar.activation(a1[:], kl[:], Copy, bias=0.0, scale=0.005)
        a2 = small.tile([1, 1], f32, tag="a2")
        nc.scalar.activation(a2[:], vs[:], Copy, bias=0.0, scale=0.003 / (E - 1))
        a3 = small.tile([1, 1], f32, tag="a3")
        nc.scalar.activation(a3[:], es[:], Copy, bias=0.002 * MAX_ENT, scale=0.002)
        lbt = small.tile([1, 1], f32, tag="lbt")
        nc.vector.tensor_add(lbt[:], a1[:], a2[:])
        nc.vector.tensor_add(lbt[:], lbt[:], a3[:])
        nc.sync.dma_start(out=lb_d, in_=lbt[:])


_NC_CACHE = {}


def get_nc():
    if "nc" not in _NC_CACHE:
        nc = bacc.Bacc("TRN2", target_bir_lowering=False, debug=False, num_devices=NC)
        with tile.TileContext(nc) as tc:
            build_moe(nc, tc)
        nc.compile()
        _NC_CACHE["nc"] = nc
    return _NC_CACHE["nc"]


def make_in_maps(inputs):
    x = np.ascontiguousarray(np.asarray(inputs["x"], np.float32))
    in_maps = []
    for c in range(NC):
        eoh = np.zeros((128, E), np.float32)
        eoh[:, c] = 1.0
        in_maps.append(
            {
                "x": x,
                "xs": np.ascontiguousarray(x[c * BS : (c + 1) * BS]),
                "gw1": np.asarray(inputs["gw1"], np.float32),
                "gb1": np.asarray(inputs["gb1"], np.float32).reshape(256, 1),
                "gw2": np.asarray(inputs["gw2"], np.float32),
                "gb2": np.asarray(inputs["gb2"], np.float32).reshape(128, 1),
                "gw3": np.asarray(inputs["gw3"], np.float32),
                "gb3": np.asarray(inputs["gb3"], np.float32).reshape(E, 1),
                "ew1": np.ascontiguousarray(np.asarray(inputs["ew1"], np.float32)[c]),
                "eb1": np.asarray(inputs["eb1"], np.float32)[c].reshape(H, 1),
                "ew2": np.ascontiguousarray(np.asarray(inputs["ew2"], np.float32)[c]),
                "eb2": np.asarray(inputs["eb2"], np.float32)[c].reshape(H, 1),
                "ew3": np.ascontiguousarray(np.asarray(inputs["ew3"], np.float32)[c]),
                "eb3": np.asarray(inputs["eb3"], np.float32)[c].reshape(O, 1),
                "eoh": eoh,
            }
        )
    return in_maps


def run(inputs, trace=False, **kw):
    nc = get_nc()
    in_maps = make_in_maps(inputs)
    res = bass_utils.run_bass_kernel_spmd(
        nc, in_maps, core_ids=list(range(NC)), trace=trace, **kw
    )
    yT = res.results[0]["yT"].astype(np.float64)
    for c in range(1, NC):
        yT += res.results[c]["yT"]
    y = np.ascontiguousarray(yT.T).astype(np.float32)
    gp = np.concatenate([res.results[c]["gp"] for c in range(NC)], axis=0)
    lb = np.float32(res.results[0]["lb"][0, 0])
    return (y, lb, gp), res


def kernel(**inputs):
    (y, lb, gp), _ = run(inputs)
    return y, lb, gp


# revision 11
# speedup vs baseline: 51.6014x; 51.6014x over previous
"""MoE layer kernel for 8 Trainium2 NeuronCores.

Strategy (phase 1, dense expert-parallel):
  - Gate network data-parallel: core c computes gate probs for tokens
    [c*1024, (c+1)*1024) in fp32 exactly (x transposed via PE), AllGather ->
    every core has full gate_probs.
  - Experts expert-parallel: core c runs expert c densely over all 8192
    tokens in bf16 (fp32 accumulate), scales by its combine weight column,
    writes a feature-major partial output yT [O, B].
  - Host: sum partials over cores, transpose -> [B, O]; concat gate prob
    shards; lb_loss computed on device (core 0).
"""

import sys

for _p in ("/opt/trn_rl_repo",):
    if _p not in sys.path:
        sys.path.insert(0, _p)

import numpy as np

import concourse.mybir as mybir
import concourse.tile as tile
from concourse import bacc, bass_utils
from concourse.masks import make_identity

B, D, H, O, E, TOPK = 8192, 1024, 2048, 1024, 8, 2
NC = 8
TB = 256  # token block
NBLK = B // TB  # 32
BS = B // NC  # tokens per core for the gate = 1024
GBLK = BS // TB  # 4

f32 = mybir.dt.float32
bf16 = mybir.dt.bfloat16
fp8 = mybir.dt.float8e4
DR = mybir.MatmulPerfMode.DoubleRow
USE_FP8 = False
EDT = fp8 if USE_FP8 else bf16
KSTEP = 2 if USE_FP8 else 1
PM = DR if USE_FP8 else None
SW = 64.0 if USE_FP8 else 1.0   # fp8 weight scale
SH = 8.0 if USE_FP8 else 1.0    # fp8 activation scale
Relu = mybir.ActivationFunctionType.Relu
Exp = mybir.ActivationFunctionType.Exp
Ln = mybir.ActivationFunctionType.Ln
Copy = mybir.ActivationFunctionType.Copy
AX = mybir.AxisListType.X

KD = D // 128  # 8   k-tiles of D
MH = H // 128  # 16  m-tiles of H
MO = O // 128  # 8   m-tiles of O

LOG8 = float(np.log(8.0))
MAX_ENT = 8.0 * LOG8


def build_moe(nc, tc, collective=True):
    dt = nc.dram_tensor
    x_d = dt("x", (B, D), f32, kind="ExternalInput").ap()
    xs_d = dt("xs", (BS, D), f32, kind="ExternalInput").ap()
    gw1_d = dt("gw1", (D, 256), f32, kind="ExternalInput").ap()
    gb1_d = dt("gb1", (256, 1), f32, kind="ExternalInput").ap()
    gw2_d = dt("gw2", (256, 128), f32, kind="ExternalInput").ap()
    gb2_d = dt("gb2", (128, 1), f32, kind="ExternalInput").ap()
    gw3_d = dt("gw3", (128, E), f32, kind="ExternalInput").ap()
    gb3_d = dt("gb3", (E, 1), f32, kind="ExternalInput").ap()
    ew1_d = dt("ew1", (D, H), f32, kind="ExternalInput").ap()
    eb1_d = dt("eb1", (H, 1), f32, kind="ExternalInput").ap()
    ew2_d = dt("ew2", (H, H), f32, kind="ExternalInput").ap()
    eb2_d = dt("eb2", (H, 1), f32, kind="ExternalInput").ap()
    ew3_d = dt("ew3", (H, O), f32, kind="ExternalInput").ap()
    eb3_d = dt("eb3", (O, 1), f32, kind="ExternalInput").ap()
    eoh_d = dt("eoh", (128, E), f32, kind="ExternalInput").ap()

    yT_d = dt("yT", (O, B), f32, kind="ExternalOutput").ap()
    gp_d = dt("gp", (BS, E), f32, kind="ExternalOutput").ap()
    lb_d = dt("lb", (1, 1), f32, kind="ExternalOutput").ap()

    with (
        tc.tile_pool(name="wpool", bufs=1) as wpool,
        tc.tile_pool(name="stage", bufs=2) as stage,
        tc.tile_pool(name="xtok", bufs=2) as xtok_p,
        tc.tile_pool(name="const", bufs=1) as const,
        tc.tile_pool(name="xt", bufs=3) as xt,
        tc.tile_pool(name="xbf", bufs=1) as xbf,
        tc.tile_pool(name="hpool", bufs=1) as hpool,
        tc.tile_pool(name="gpool", bufs=1) as gpool,
        tc.tile_pool(name="small", bufs=4) as small,
        tc.tile_pool(name="ypool", bufs=3) as ypool,
        tc.tile_pool(name="mm", bufs=4, space="PSUM") as mmp,
        tc.tile_pool(name="gps", bufs=2, space="PSUM") as gpsp,
        tc.tile_pool(name="tp", bufs=2, space="PSUM") as tpp,
        tc.tile_pool(name="dram", bufs=1, space="DRAM") as dram,
    ):
        # ---------------- constants ----------------
        ident = const.tile([128, 128], f32)
        make_identity(nc, ident[:])
        ones = const.tile([128, 1], f32)
        nc.vector.memset(ones[:], 1.0)
        eoh = const.tile([128, E], f32)
        nc.sync.dma_start(out=eoh[:], in_=eoh_d)
        ones_row = const.tile([1, 128], f32)
        nc.vector.memset(ones_row[:], 1.0)
        usage = const.tile([1, E], f32)
        nc.vector.memset(usage[:], 0.0)
        eps_c = const.tile([1, 1], f32)
        nc.vector.memset(eps_c[:], 1e-8)

        # ---------------- expert weights -> bf16 resident ----------------
        w1 = wpool.tile([128, KD, H], EDT)
        w2 = wpool.tile([128, MH, H], EDT)
        w3 = wpool.tile([128, MH, O], EDT)
        def load_w_fp8(dst, src_d, k, width):
            for h in range(0, width, 1024):
                st = stage.tile([128, 1024], f32, tag="stage")
                nc.sync.dma_start(
                    out=st[:], in_=src_d[k * 128 : (k + 1) * 128, h : h + 1024]
                )
                nc.scalar.mul(dst[:, k, h : h + 1024], st[:], SW)

        for k in range(KD):
            load_w_fp8(w1, ew1_d, k, H)
        for k in range(MH):
            load_w_fp8(w2, ew2_d, k, H)
        for k in range(MH):
            load_w_fp8(w3, ew3_d, k, O)

        # gate weights fp32 resident
        g1w = wpool.tile([128, KD, 256], f32)
        for k in range(KD):
            nc.sync.dma_start(out=g1w[:, k, :], in_=gw1_d[k * 128 : (k + 1) * 128, :])
        g2w = wpool.tile([128, 2, 128], f32)
        for k in range(2):
            nc.sync.dma_start(out=g2w[:, k, :], in_=gw2_d[k * 128 : (k + 1) * 128, :])
        g3w = wpool.tile([128, E], f32)
        nc.sync.dma_start(out=g3w[:], in_=gw3_d)

        # biases as per-partition columns
        def load_bias_cols(dram_ap, n_m, nm):
            t = const.tile([128, n_m], f32, tag=f"bias_{nm}")
            nc.sync.dma_start(
                out=t[:], in_=dram_ap.rearrange("(m p) o -> p (m o)", p=128)
            )
            return t

        eb1s = load_bias_cols(eb1_d, MH, "eb1")
        eb2s = load_bias_cols(eb2_d, MH, "eb2")
        eb3s = load_bias_cols(eb3_d, MO, "eb3")
        gb1s = load_bias_cols(gb1_d, 2, "gb1")
        gb2s = load_bias_cols(gb2_d, 1, "gb2")
        gb3s = const.tile([E, 1], f32)
        nc.sync.dma_start(out=gb3s[:], in_=gb3_d)
        eb1sS = const.tile([128, MH], f32)
        nc.scalar.mul(eb1sS[:], eb1s[:], SH)
        eb2sS = const.tile([128, MH], f32)
        nc.scalar.mul(eb2sS[:], eb2s[:], SH)

        xbf_dram = dram.tile([B, D], bf16)  # bf16 copy of x
        gpl = dram.tile([BS, E], f32)  # gate probs local (collective in)
        gpa = dram.tile([B, E], f32)  # gate probs allgathered

        # ---------------- cast pass: x -> bf16 DRAM ----------------
        for i in range(B // 128):
            st = stage.tile([128, D], f32, tag="stage")
            nc.sync.dma_start(out=st[:, :D], in_=x_d[i * 128 : (i + 1) * 128, :])
            cb = stage.tile([128, D], bf16, tag="stage")
            nc.vector.tensor_copy(cb[:], st[:, :D])
            nc.sync.dma_start(out=xbf_dram[i * 128 : (i + 1) * 128, :], in_=cb[:])

        # ---------------- gate phase (core's token slice, fp32) ----------------
        for blk in range(GBLK):
            t0 = blk * TB
            xtoks = []
            for j in range(TB // 128):
                xj = xtok_p.tile([128, D], f32, tag="xtok")
                nc.sync.dma_start(out=xj[:], in_=xs_d[t0 + j * 128 : t0 + (j + 1) * 128, :])
                xtoks.append(xj)
            ps0 = gpsp.tile([128, TB], f32, tag="gps")
            ps1 = gpsp.tile([128, TB], f32, tag="gps")
            for k in range(KD):
                xk32 = xt.tile([128, TB], f32, tag="xt32")
                for j in range(TB // 128):
                    tps = tpp.tile([128, 128], f32, tag="tp")
                    nc.tensor.transpose(
                        tps[:], xtoks[j][:, k * 128 : (k + 1) * 128], ident[:]
                    )
                    nc.vector.tensor_copy(xk32[:, j * 128 : (j + 1) * 128], tps[:])
                nc.tensor.matmul(
                    ps0[:], g1w[:, k, 0:128], xk32[:],
                    start=(k == 0), stop=(k == KD - 1),
                )
                nc.tensor.matmul(
                    ps1[:], g1w[:, k, 128:256], xk32[:],
                    start=(k == 0), stop=(k == KD - 1),
                )
            g1 = gpool.tile([128, 2, TB], f32, tag="g1")
            nc.scalar.activation(g1[:, 0, :], ps0[:], Relu, bias=gb1s[:, 0:1])
            nc.scalar.activation(g1[:, 1, :], ps1[:], Relu, bias=gb1s[:, 1:2])
            ps2 = mmp.tile([128, TB], f32, tag="mm")
            nc.tensor.matmul(ps2[:], g2w[:, 0, :], g1[:, 0, :], start=True, stop=False)
            nc.tensor.matmul(ps2[:], g2w[:, 1, :], g1[:, 1, :], start=False, stop=True)
            g2 = gpool.tile([128, TB], f32, tag="g2")
            nc.scalar.activation(g2[:], ps2[:], Relu, bias=gb2s[:, 0:1])
            ps3 = mmp.tile([E, TB], f32, tag="mm")
            nc.tensor.matmul(ps3[:], g3w[:], g2[:], start=True, stop=True)
            logits = gpool.tile([E, TB], f32, tag="logits")
            nc.vector.tensor_scalar_add(logits[:], ps3[:], gb3s[:, 0:1])

            for j in range(TB // 128):
                tp = tpp.tile([128, E], f32, tag="tp")
                nc.tensor.transpose(
                    tp[:], logits[:, j * 128 : (j + 1) * 128], ident[0:E, 0:E]
                )
                lgt = small.tile([128, E], f32, tag="lgt")
                nc.vector.tensor_copy(lgt[:], tp[:])
                s8 = small.tile([128, 8], f32, tag="s8")
                nc.vector.max(s8[:], lgt[:])
                nm = small.tile([128, 1], f32, tag="nm")
                nc.scalar.mul(nm[:], s8[:, 0:1], -1.0)
                ex = small.tile([128, E], f32, tag="ex")
                nc.scalar.activation(ex[:], lgt[:], Exp, bias=nm[:, 0:1])
                ssum = small.tile([128, 1], f32, tag="ssum")
                nc.vector.reduce_sum(ssum[:], ex[:], axis=AX)
                rs = small.tile([128, 1], f32, tag="rs")
                nc.vector.reciprocal(rs[:], ssum[:])
                pt = small.tile([128, E], f32, tag="pt")
                nc.vector.tensor_scalar_mul(pt[:], ex[:], rs[:, 0:1])
                r0 = t0 + j * 128
                nc.sync.dma_start(out=gp_d[r0 : r0 + 128, :], in_=pt[:])
                nc.sync.dma_start(out=gpl[r0 : r0 + 128, :], in_=pt[:])

        if collective:
            nc.gpsimd.collective_compute(
                "AllGather",
                mybir.AluOpType.bypass,
                replica_groups=[list(range(NC))],
                ins=[gpl[:].opt()],
                outs=[gpa[:].opt()],
            )
        else:
            # single-core timing variant: stand-in DMA with similar traffic
            nc.sync.dma_start(out=gpa[0:BS, :], in_=gpl[:])

        # ---------------- expert phase (all tokens, bf16) ----------------
        for blk in range(NBLK):
            t0 = blk * TB
            # combine-weight row for this block (+ usage accumulation)
            ccps = tpp.tile([1, TB], f32, tag="tp")
            for j in range(TB // 128):
                r0 = t0 + j * 128
                pt = small.tile([128, E], f32, tag="cpt")
                nc.sync.dma_start(out=pt[:], in_=gpa[r0 : r0 + 128, :])
                s8 = small.tile([128, 8], f32, tag="cs8")
                nc.vector.max(s8[:], pt[:])
                den = small.tile([128, 1], f32, tag="cden")
                nc.vector.tensor_add(den[:], s8[:, 0:1], s8[:, 1:2])
                rden = small.tile([128, 1], f32, tag="crden")
                nc.vector.reciprocal(rden[:], den[:])
                mask = small.tile([128, E], f32, tag="cmask")
                nc.vector.tensor_scalar(
                    out=mask[:], in0=pt[:], scalar1=s8[:, 1:2], scalar2=None,
                    op0=mybir.AluOpType.is_ge,
                )
                cw = small.tile([128, E], f32, tag="ccw")
                nc.vector.tensor_mul(cw[:], pt[:], mask[:])
                nc.vector.tensor_scalar_mul(cw[:], cw[:], rden[:, 0:1])
                nc.vector.tensor_mul(cw[:], cw[:], eoh[:])
                cc = small.tile([128, 1], f32, tag="ccc")
                nc.vector.reduce_sum(cc[:], cw[:], axis=AX)
                nc.tensor.transpose(ccps[0:1, j * 128 : (j + 1) * 128], cc[:], ident[:])
                ups = tpp.tile([1, E], f32, tag="tp")
                nc.tensor.matmul(ups[:], ones[:], pt[:], start=True, stop=True)
                nc.vector.tensor_add(usage[:], usage[:], ups[:])
            ccrow = small.tile([1, TB], f32, tag="ccrow")
            nc.vector.tensor_copy(ccrow[:], ccps[:])
            ccrep_ps = tpp.tile([128, TB], f32, tag="tp")
            nc.tensor.matmul(ccrep_ps[:], ones_row[:], ccrow[:], start=True, stop=True)
            ccrep = small.tile([128, TB], f32, tag="ccrep")
            nc.vector.tensor_copy(ccrep[:], ccrep_ps[:])

            # x^T block (bf16) via xbar transpose from xbf_dram
            if USE_FP8:
                xbh = xbf.tile([128, KD, TB], bf16, tag="xbh")
                xb = xbf.tile([128, KD, TB], fp8, tag="xb")
            else:
                xb = xbf.tile([128, KD, TB], bf16, tag="xb")
                xbh = xb
            for k in range(KD):
                nc.sync.dma_start(
                    out=xbh[:, k, :],
                    in_=xbf_dram[t0 : t0 + TB, k * 128 : (k + 1) * 128],
                    transpose=True,
                )
                if USE_FP8:
                    nc.vector.tensor_copy(xb[:, k, :], xbh[:, k, :])

            h1 = hpool.tile([128, MH, TB], EDT, tag="h1")
            for m in range(MH):
                ps = mmp.tile([128, TB], f32, tag="mm")
                for k in range(0, KD, KSTEP):
                    nc.tensor.matmul(
                        ps[:], w1[:, k : k + KSTEP, m * 128 : (m + 1) * 128],
                        xb[:, k : k + KSTEP, :],
                        start=(k == 0), stop=(k == KD - KSTEP), perf_mode=PM,
                    )
                nc.scalar.activation(
                    h1[:, m, :], ps[:], Relu, bias=eb1sS[:, m : m + 1], scale=SH / SW
                )
            h2 = hpool.tile([128, MH, TB], EDT, tag="h2")
            for m in range(MH):
                ps = mmp.tile([128, TB], f32, tag="mm")
                for k in range(0, MH, KSTEP):
                    nc.tensor.matmul(
                        ps[:], w2[:, k : k + KSTEP, m * 128 : (m + 1) * 128],
                        h1[:, k : k + KSTEP, :],
                        start=(k == 0), stop=(k == MH - KSTEP), perf_mode=PM,
                    )
                nc.scalar.activation(
                    h2[:, m, :], ps[:], Relu, bias=eb2sS[:, m : m + 1],
                    scale=SH / (SW * SH),
                )
            for m in range(MO):
                ps = mmp.tile([128, TB], f32, tag="mm")
                for k in range(0, MH, KSTEP):
                    nc.tensor.matmul(
                        ps[:], w3[:, k : k + KSTEP, m * 128 : (m + 1) * 128],
                        h2[:, k : k + KSTEP, :],
                        start=(k == 0), stop=(k == MH - KSTEP), perf_mode=PM,
                    )
                yt = ypool.tile([128, TB], f32, tag="yt")
                nc.vector.tensor_scalar(
                    out=yt[:], in0=ps[:], scalar1=1.0 / (SW * SH),
                    scalar2=eb3s[:, m : m + 1],
                    op0=mybir.AluOpType.mult, op1=mybir.AluOpType.add,
                )
                nc.vector.tensor_mul(yt[:], yt[:], ccrep[:])
                nc.sync.dma_start(
                    out=yT_d[m * 128 : (m + 1) * 128, t0 : t0 + TB], in_=yt[:]
                )

        # ---------------- lb loss ----------------
        u = small.tile([1, E], f32, tag="u")
        nc.scalar.mul(u[:], usage[:], 1.0 / B)
        lu = small.tile([1, E], f32, tag="lu")
        nc.scalar.activation(lu[:], u[:], Ln, bias=eps_c[:, 0:1])
        kls = small.tile([1, 1], f32, tag="kls")
        nc.vector.reduce_sum(kls[:], lu[:], axis=AX)
        kl = small.tile([1, 1], f32, tag="kl")
        # kl = log(1/8) - mean(log(u))
        nc.scalar.activation(kl[:], kls[:], Copy, bias=-LOG8, scale=-1.0 / E)
        # ent = -sum(u * log u); contribution 0.2*0.01*(max_ent - ent)
        ulu = small.tile([1, E], f32, tag="ulu")
        nc.vector.tensor_mul(ulu[:], u[:], lu[:])
        es = small.tile([1, 1], f32, tag="es")
        nc.vector.reduce_sum(es[:], ulu[:], axis=AX)
        # var (ddof=1)
        usum = small.tile([1, 1], f32, tag="usum")
        nc.vector.reduce_sum(usum[:], u[:], axis=AX)
        um = small.tile([1, 1], f32, tag="um")
        nc.scalar.mul(um[:], usum[:], 1.0 / E)
        dv = small.tile([1, E], f32, tag="dv")
        nc.vector.tensor_scalar_sub(dv[:], u[:], um[:, 0:1])
        nc.vector.tensor_mul(dv[:], dv[:], dv[:])
        vs = small.tile([1, 1], f32, tag="vs")
        nc.vector.reduce_sum(vs[:], dv[:], axis=AX)

        a1 = small.tile([1, 1], f32, tag="a1")
        nc.scalar.activation(a1[:], kl[:], Copy, bias=0.0, scale=0.005)
        a2 = small.tile([1, 1], f32, tag="a2")
        nc.scalar.activation(a2[:], vs[:], Copy, bias=0.0, scale=0.003 / (E - 1))
        a3 = small.tile([1, 1], f32, tag="a3")
        nc.scalar.activation(a3[:], es[:], Copy, bias=0.002 * MAX_ENT, scale=0.002)
        lbt = small.tile([1, 1], f32, tag="lbt")
        nc.vector.tensor_add(lbt[:], a1[:], a2[:])
        nc.vector.tensor_add(lbt[:], lbt[:], a3[:])
        nc.sync.dma_start(out=lb_d, in_=lbt[:])


_NC_CACHE = {}


def get_nc(single_core=False):
    key = "nc1" if single_core else "nc"
    if key not in _NC_CACHE:
        nc = bacc.Bacc(
            "TRN2", target_bir_lowering=False, debug=False,
            num_devices=1 if single_core else NC,
        )
        with tile.TileContext(nc) as tc:
            build_moe(nc, tc, collective=not single_core)
        nc.compile()
        _NC_CACHE[key] = nc
    return _NC_CACHE[key]


def make_in_maps(inputs):
    x = np.ascontiguousarray(np.asarray(inputs["x"], np.float32))
    in_maps = []
    for c in range(NC):
        eoh = np.zeros((128, E), np.float32)
        eoh[:, c] = 1.0
        in_maps.append(
            {
                "x": x,
                "xs": np.ascontiguousarray(x[c * BS : (c + 1) * BS]),
                "gw1": np.asarray(inputs["gw1"], np.float32),
                "gb1": np.asarray(inputs["gb1"], np.float32).reshape(256, 1),
                "gw2": np.asarray(inputs["gw2"], np.float32),
                "gb2": np.asarray(inputs["gb2"], np.float32).reshape(128, 1),
                "gw3": np.asarray(inputs["gw3"], np.float32),
                "gb3": np.asarray(inputs["gb3"], np.float32).reshape(E, 1),
                "ew1": np.ascontiguousarray(np.asarray(inputs["ew1"], np.float32)[c]),
                "eb1": np.asarray(inputs["eb1"], np.float32)[c].reshape(H, 1),
                "ew2": np.ascontiguousarray(np.asarray(inputs["ew2"], np.float32)[c]),
                "eb2": np.asarray(inputs["eb2"], np.float32)[c].reshape(H, 1),
                "ew3": np.ascontiguousarray(np.asarray(inputs["ew3"], np.float32)[c]),
                "eb3": np.asarray(inputs["eb3"], np.float32)[c].reshape(O, 1),
                "eoh": eoh,
            }
        )
    return in_maps


def run(inputs, trace=False, **kw):
    nc = get_nc()
    in_maps = make_in_maps(inputs)
    res = bass_utils.run_bass_kernel_spmd(
        nc, in_maps, core_ids=list(range(NC)), trace=trace, **kw
    )
    yT = res.results[0]["yT"].astype(np.float64)
    for c in range(1, NC):
        yT += res.results[c]["yT"]
    y = np.ascontiguousarray(yT.T).astype(np.float32)
    gp = np.concatenate([res.results[c]["gp"] for c in range(NC)], axis=0)
    lb = np.float32(res.results[0]["lb"][0, 0])
    return (y, lb, gp), res


def kernel(**inputs):
    (y, lb, gp), _ = run(inputs)
    return y, lb, gp
